# revision 88
# baseline (speedup 1.0000x reference)
"""Trainium2 Bass kernel for nn_AutoEncoderV4 (PointNet autoencoder, 8-core data parallel).

Strategy (hardcoded for shapes R=4,B=32,N=512,D=3,L=64, 8 cores):
  - shard batch B across 8 cores (4 local b each); each core handles all R
    rotations of its b's, so the crc loss pairs are core-local.
  - rotation folded into first-layer weights (host): W0_r = rm[r]^T @ cw0.
  - LayerNorm: weights are mean-centered on host (kills mean subtraction,
    biases/gamma/beta are identity in this model -- asserted), and the
    rsqrt(var+eps) scale is propagated lazily through matmuls as a per-point
    scalar (relu commutes with positive scales). Variance comes from an ACT
    Square pass with accum_out.
  - activations bf16; matmul accum f32 in PSUM; per-layer activation
    transposes via xbar DMA transpose (bf16, SBUF->SBUF).
  - overlap loss: dist matrix per cloud via one K=5 matmul
    rows ([-2px,-2py,-2pz, sq, ones] x [px,py,pz, ones, sq]) = full dist,
    diagonal masked by adding BIG*I, row-min, relu(2r - min), summed.
  - quat rotate: q from t-branch, R(q) built on device, applied via 3x3 matmuls.
  - losses are partial sums per core; host divides by global counts.
"""

import numpy as np
import ml_dtypes

R, B, N, D, L = 4, 32, 512, 3, 64
NCORES = 8
BL = B // NCORES           # local batch per core = 4
NCL = BL * R               # e-clouds per core = 16
EPS = 1e-3
TWO_R = 1.0
BIG = 1000.0
CONV_DIMS = [3, 64, 128, 256, 512]

_CACHE = {}


def _center(w):
    return w - w.mean(axis=1, keepdims=True)


def _bf(x):
    return np.ascontiguousarray(np.asarray(x, dtype=np.float32)).astype(np.float16)


def _f32(x):
    return np.ascontiguousarray(np.asarray(x, dtype=np.float32))


def _build_nc():
    import concourse.mybir as mybir
    from concourse import bacc
    from concourse.tile import TileContext
    from contextlib import ExitStack

    bf = mybir.dt.float16
    f32 = mybir.dt.float32
    AF = mybir.ActivationFunctionType
    OP = mybir.AluOpType

    nc = bacc.Bacc()

    def din(name, shape, dt=bf):
        return nc.declare_dram_parameter(name, list(shape), dt, isOutput=False)

    # inputs
    pcT = din("pcT", [3, BL * N])
    ew0 = din("ew0", [3, 4 * 64])
    ew1 = din("ew1", [128, 128])
    ew2 = din("ew2", [128, 256])
    ew3 = din("ew3", [128, 2, 512])
    ehw0 = din("ehw0", [128, 4, 256])
    ehw1 = din("ehw1", [128, 2, 128])
    ehow = din("ehow", [128, 64])
    tw0 = din("tw0", [3, 64])
    tw1 = din("tw1", [128, 128])
    tw2 = din("tw2", [128, 256])
    tw3 = din("tw3", [128, 2, 512])
    thw0 = din("thw0", [128, 4, 256])
    thw1 = din("thw1", [128, 2, 128])
    thow = din("thow", [128, 4])
    dw0 = din("dw0", [64, 16])
    dw1 = din("dw1", [16, 32])
    dw2 = din("dw2", [32, 36])
    dw3 = din("dw3", [36, 3 * 512])
    bigi = din("bigi", [128, 128], f32)
    sel = din("sel", [16, 4], f32)
    id4 = din("id4", [4, 4], f32)
    ones128 = din("ones128", [128, 1], f32)
    ones512 = din("ones512", [1, 512], f32)
    id128h = din("id128h", [128, 128])
    ide = din("ide", [128, 896], f32)
    onesh = din("onesh", [128, 1])
    sel2 = din("sel2", [128, 2])

    rot_out = nc.declare_dram_parameter("rot_out", [NCL, 3, N], f32, isOutput=True)
    loss_out = nc.declare_dram_parameter("loss_out", [1, 2], f32, isOutput=True)

    with ExitStack() as ctx:
        tc = ctx.enter_context(TileContext(nc))
        wpool = ctx.enter_context(tc.tile_pool(name="weights", bufs=1))
        const = ctx.enter_context(tc.tile_pool(name="const", bufs=1))
        convps = ctx.enter_context(tc.tile_pool(name="convps", bufs=3, space="PSUM"))
        tpsum = ctx.enter_context(tc.tile_pool(name="tpsum", bufs=1, space="PSUM"))
        stps = ctx.enter_context(tc.tile_pool(name="stps", bufs=1, space="PSUM"))
        smallps = ctx.enter_context(tc.tile_pool(name="smallps", bufs=1, space="PSUM"))
        gramps = ctx.enter_context(tc.tile_pool(name="gramps", bufs=2, space="PSUM"))
        apool = ctx.enter_context(tc.tile_pool(name="apool", bufs=10))
        atpool = ctx.enter_context(tc.tile_pool(name="atpool", bufs=1))
        a4pool = ctx.enter_context(tc.tile_pool(name="a4pool", bufs=40))
        sqpool = ctx.enter_context(tc.tile_pool(name="sqpool", bufs=4))
        stpool = ctx.enter_context(tc.tile_pool(name="stats", bufs=40))
        mpool = ctx.enter_context(tc.tile_pool(name="misc", bufs=8))
        pcpool = ctx.enter_context(tc.tile_pool(name="pcl", bufs=12))

        def sb(pool, shape, dt=bf, tag=None, bufs=None):
            return pool.tile(list(shape), dt, tag=tag or "t", name=tag or "t",
                             bufs=bufs)

        # ---------------- load weights/constants into SBUF ----------------
        def load(dram, shape, dt=bf):
            t = sb(wpool, shape, dt, tag=dram.name)
            nc.gpsimd.dma_start(out=t[...], in_=dram[...])
            return t

        def load2(dram, shape, dt=bf):
            t = sb(wpool, shape, dt, tag=dram.name)
            nc.sync.dma_start(out=t[...], in_=dram[...])
            return t

        pcT_sb = load(pcT, [3, BL * N])
        tw0_sb = load2(tw0, [3, 64])
        tw1_sb = load(tw1, [128, 128])
        tw2_sb = load2(tw2, [128, 256])
        tw3_sb = load(tw3, [128, 2, 512])
        id128h_sb = load2(id128h, [128, 128])
        thw0_sb = load(thw0, [128, 4, 256])
        thw1_sb = load2(thw1, [128, 2, 128])
        thow_sb = load(thow, [128, 4])
        ew0_sb = load2(ew0, [3, 256])
        ew1_sb = load(ew1, [128, 128])
        ew2_sb = load2(ew2, [128, 256])
        ew3_sb = load(ew3, [128, 2, 512])
        ehw0_sb = load2(ehw0, [128, 4, 256])
        ehw1_sb = load(ehw1, [128, 2, 128])
        ehow_sb = load2(ehow, [128, 64])
        dw0_sb = load(dw0, [64, 16])
        dw1_sb = load2(dw1, [16, 32])
        dw2_sb = load(dw2, [32, 36])
        dw3_sb = load2(dw3, [36, 3 * 512])
        bigi_sb = load(bigi, [128, 128], f32)
        sel_sb = load2(sel, [16, 4], f32)
        id4_sb = load(id4, [4, 4], f32)
        onesh_sb = load(onesh, [128, 1])
        sel2_sb = load2(sel2, [128, 2])
        ones_sb = load(ones128, [128, 1], f32)

        enc_all = const.tile([16, 64], f32)     # assembled e encodings
        minds = const.tile([128, 64], f32)      # row-mins of dist, col = 4*c + i
        ovcols = const.tile([128, 4], f32)      # per-group overlap partial columns
        RT = const.tile([3, 12], f32)           # R(q)^T per local b

        # ---------------- shared helpers ----------------
        def relu_evict(dst, src):
            nc.vector.tensor_scalar_max(dst, src, 0.0)

        def conv_group(ws, col_of, nch=16, mid_cb=None):
            """One group of nch conv chunks. ws = (w0rhs_fn, w1, w2, w3).
            col_of(ci) -> column offset into pcT_sb. Returns (a4_list, s4)."""
            w0rhs, w1_sb, w2_sb, w3_sb = ws
            tcols = [sb(stpool, [128, nch], f32, tag="tstat") for _ in range(4)]
            a4s = [None] * nch
            for wv in range(nch // 8):
                if wv == 1 and mid_cb is not None:
                    mid_cb()
                cis = list(range(8 * wv, 8 * wv + 8))

                def stats_mm(sq_tiles, kdims, tcol):
                    stp = sb(stps, [128, 1], f32, tag="stps")
                    for si, (sq_t, kd) in enumerate(zip(sq_tiles, kdims)):
                        nc.tensor.matmul(stp[...], sq_t[...], onesh_sb[0:kd, 0:1],
                                         start=(si == 0), stop=(si == len(sq_tiles) - 1))
                    return stp

                # L1 (layout B, two chunks packed per psum tile at bases 0/64)
                hp1 = {}
                for j in range(4):
                    c0, c1 = cis[2 * j], cis[2 * j + 1]
                    t1 = sb(convps, [128, 128], f32, tag="convps")
                    nc.tensor.matmul(t1[0:64, :], w0rhs(c0),
                                     pcT_sb[:, col_of(c0):col_of(c0) + 128],
                                     start=True, stop=True)
                    nc.tensor.matmul(t1[64:128, :], w0rhs(c1),
                                     pcT_sb[:, col_of(c1):col_of(c1) + 128],
                                     start=True, stop=True)
                    hp1[j] = t1
                sqd = {}
                for j in range(4):
                    sq = sb(sqpool, [128, 128], bf, tag="sqB1", bufs=10)
                    nc.scalar.activation(sq[...], hp1[j][...], AF.Square)
                    sqd[j] = sq
                acts1 = {}
                for j in range(4):
                    a1 = sb(apool, [128, 128], bf, tag="a1")
                    nc.vector.tensor_scalar_max(a1[...], hp1[j][...], 0.0)
                    acts1[j] = a1
                for j in range(4):
                    c0 = cis[2 * j]
                    stp = sb(stps, [128, 2], f32, tag="stps")
                    nc.tensor.matmul(stp[...], sqd[j][...], sel2_sb[...],
                                     start=True, stop=True)
                    if j % 2 == 0:
                        nc.scalar.copy(tcols[0][:, c0:c0 + 2], stp[...])
                    else:
                        nc.vector.tensor_copy(tcols[0][:, c0:c0 + 2], stp[...])
                # L2
                hp = {}
                for ci in cis:
                    j, par = (ci - cis[0]) // 2, ci % 2
                    hp[ci] = sb(convps, [128, 128], f32, tag="convps")
                    if par == 0:
                        nc.tensor.matmul(hp[ci][...], w1_sb[0:64, :],
                                         acts1[j][0:64, :], start=True, stop=True)
                    else:
                        nc.tensor.matmul(hp[ci][...], w1_sb[64:128, :],
                                         acts1[j][64:128, :], start=True, stop=True)
                sqd = {}
                for ci in cis:
                    sq = sb(sqpool, [128, 128], bf, tag="sqB2", bufs=10)
                    nc.scalar.activation(sq[...], hp[ci][...], AF.Square)
                    sqd[ci] = sq
                acts = {}
                for ci in cis:
                    a2 = sb(apool, [128, 128], bf, tag="a2")
                    nc.vector.tensor_scalar_max(a2[...], hp[ci][...], 0.0)
                    acts[ci] = a2
                for ci in cis:
                    stp = stats_mm([sqd[ci]], [128], None)
                    if ci % 2 == 0:
                        nc.scalar.copy(tcols[1][:, ci:ci + 1], stp[...])
                    else:
                        nc.vector.tensor_copy(tcols[1][:, ci:ci + 1], stp[...])
                # L3 (two M-chunks packed into one [128,256] psum tile)
                hp3 = {}
                for ci in cis:
                    t3 = sb(convps, [128, 256], f32, tag="convps")
                    nc.tensor.matmul(t3[:, 0:128], w2_sb[:, 0:128], acts[ci][...],
                                     start=True, stop=True)
                    nc.tensor.matmul(t3[:, 128:256], w2_sb[:, 128:256], acts[ci][...],
                                     start=True, stop=True)
                    hp3[ci] = t3
                sq3 = {}
                for ci in cis:
                    sq = sb(sqpool, [128, 256], bf, tag="sqB3", bufs=10)
                    nc.scalar.activation(sq[...], hp3[ci][...], AF.Square)
                    sq3[ci] = sq
                a3 = {}
                for ci in cis:
                    aa = sb(apool, [128, 256], bf, tag="a3")
                    nc.vector.tensor_scalar_max(aa[...], hp3[ci][...], 0.0)
                    a3[ci] = aa
                for ci in cis:
                    stp = sb(stps, [128, 1], f32, tag="stps")
                    nc.tensor.matmul(stp[...], sq3[ci][:, 0:128], onesh_sb[...],
                                     start=True, stop=False)
                    nc.tensor.matmul(stp[...], sq3[ci][:, 128:256], onesh_sb[...],
                                     start=False, stop=True)
                    if ci % 2 == 0:
                        nc.scalar.copy(tcols[2][:, ci:ci + 1], stp[...])
                    else:
                        nc.vector.tensor_copy(tcols[2][:, ci:ci + 1], stp[...])
                # L4 (layout A: lhsT = a3 tiles; out [128 pts, 512])
                hp = {}
                for ci in cis:
                    hp[ci] = sb(convps, [128, 512], f32, tag="convps")
                    nc.tensor.matmul(hp[ci][...], a3[ci][:, 0:128], w3_sb[:, 0, :],
                                     start=True, stop=False)
                    nc.tensor.matmul(hp[ci][...], a3[ci][:, 128:256], w3_sb[:, 1, :],
                                     start=False, stop=True)
                for ci in cis:
                    sqs = sb(sqpool, [128, 512], bf, tag="sqs")
                    nc.scalar.activation(sqs[:, 0:512], hp[ci][...], AF.Square,
                                         accum_out=tcols[3][:, ci:ci + 1])
                for ci in cis:
                    a4 = sb(a4pool, [128, 512], bf, tag="a4")
                    nc.vector.tensor_scalar_max(a4[...], hp[ci][...], 0.0)
                    a4s[ci] = a4
            # scalar batch: s_l = s_{l-1} * rsqrt(s_{l-1}^2 * t_l/F + eps)
            s_prev = None
            s_l = None
            for l in range(4):
                F = float(CONV_DIMS[l + 1])
                u = sb(stpool, [128, nch], f32, tag="tstat")
                if s_prev is None:
                    nc.vector.tensor_scalar(u[...], tcols[l][...], 1.0 / F, EPS,
                                            OP.mult, OP.add)
                else:
                    s2 = sb(stpool, [128, nch], f32, tag="tstat")
                    nc.vector.tensor_tensor(s2[...], s_prev[...], s_prev[...], OP.mult)
                    nc.vector.tensor_scalar_mul(s2[...], s2[...], 1.0 / F)
                    nc.vector.tensor_tensor(s2[...], s2[...], tcols[l][...], OP.mult)
                    nc.vector.tensor_scalar_add(u[...], s2[...], EPS)
                w = sb(stpool, [128, nch], f32, tag="tstat")
                nc.scalar.activation(w[...], u[...], AF.Sqrt)
                wi = sb(stpool, [128, nch], f32, tag="tstat")
                nc.vector.reciprocal(wi[...], w[...])
                if s_prev is None:
                    s_l = wi
                else:
                    s_l = sb(stpool, [128, nch], f32, tag="tstat")
                    nc.vector.tensor_tensor(s_l[...], s_prev[...], wi[...], OP.mult)
                s_prev = s_l
            return a4s, s_l

        def pool_cloud(a4s, s4, cis):
            """Scaled max over the 4 chunks cis; returns list of 4 gT [128,1] APs
            appended into caller-provided gT tiles."""
            vs = []
            for ci in cis:
                v = sb(a4pool, [128, 512], bf, tag="a4")
                nc.scalar.mul(v[...], a4s[ci][...], s4[:, ci:ci + 1])
                vs.append(v)
            m = vs[0]
            for v in vs[1:]:
                nc.vector.tensor_tensor(m[...], m[...], v[...], OP.max)
            return m

        def pool_reduce(m, gTs, cslot):
            for k in range(4):
                tpp = sb(tpsum, [128, 128], bf, tag="tp")
                nc.tensor.transpose(tpp[...], m[:, 128 * k:128 * (k + 1)],
                                    id128h_sb[...])
                nc.vector.tensor_reduce(gTs[k][:, cslot:cslot + 1], tpp[...],
                                        mybir.AxisListType.X, OP.max)

        def head(gTs, gsl, hw0_sb, hw1_sb, how_sb, lout, nclg):
            """gTs: 4 tiles [128, >=nclg] bf; gsl: slice cols 0:nclg.
            Returns psum [nclg, lout] encoding (pre-scale) and s6 [nclg,1]."""
            h5 = sb(smallps, [nclg, 256], f32, tag="smallps")
            for k in range(4):
                nc.tensor.matmul(h5[...], gTs[k][:, 0:nclg], hw0_sb[:, k, :],
                                 start=(k == 0), stop=(k == 3))
            t5 = sb(stpool, [nclg, 1], f32, tag="hstat")
            sqs = sb(sqpool, [128, 512], bf, tag="sqs")
            nc.scalar.activation(sqs[0:nclg, 0:256], h5[...], AF.Square,
                                 accum_out=t5[...])
            u = sb(stpool, [nclg, 1], f32, tag="hstat")
            nc.vector.tensor_scalar(u[...], t5[...], 1.0 / 256.0, EPS, OP.mult, OP.add)
            w5 = sb(stpool, [nclg, 1], f32, tag="hstat")
            nc.scalar.activation(w5[...], u[...], AF.Sqrt)
            s5 = sb(stpool, [nclg, 1], f32, tag="hstat")
            nc.vector.reciprocal(s5[...], w5[...])
            a5 = sb(mpool, [4, 256], bf, tag="a5")
            relu_evict(a5[...], h5[...])
            a5T = sb(mpool, [128, 2, 4], bf, tag="a5T")
            for kk in range(2):
                tph = sb(tpsum, [128, 128], bf, tag="tp")
                nc.tensor.transpose(tph[0:128, 0:4], a5[:, 128 * kk:128 * (kk + 1)],
                                    id128h_sb[0:4, 0:4])
                nc.vector.tensor_copy(a5T[:, kk, :], tph[0:128, 0:4])

            h6 = sb(smallps, [nclg, 128], f32, tag="smallps")
            for k in range(2):
                nc.tensor.matmul(h6[...], a5T[:, k, 0:nclg], hw1_sb[:, k, :],
                                 start=(k == 0), stop=(k == 1))
            t6 = sb(stpool, [nclg, 1], f32, tag="hstat")
            sqs = sb(sqpool, [128, 512], bf, tag="sqs")
            nc.scalar.activation(sqs[0:nclg, 0:128], h6[...], AF.Square,
                                 accum_out=t6[...])
            u6 = sb(stpool, [nclg, 1], f32, tag="hstat")
            nc.vector.tensor_tensor(u6[...], s5[...], s5[...], OP.mult)
            nc.vector.tensor_scalar_mul(u6[...], u6[...], 1.0 / 128.0)
            nc.vector.tensor_tensor(u6[...], u6[...], t6[...], OP.mult)
            nc.vector.tensor_scalar_add(u6[...], u6[...], EPS)
            w6 = sb(stpool, [nclg, 1], f32, tag="hstat")
            nc.scalar.activation(w6[...], u6[...], AF.Sqrt)
            r6 = sb(stpool, [nclg, 1], f32, tag="hstat")
            nc.vector.reciprocal(r6[...], w6[...])
            s6 = sb(stpool, [nclg, 1], f32, tag="hstat")
            nc.vector.tensor_tensor(s6[...], s5[...], r6[...], OP.mult)
            a6 = sb(mpool, [4, 128], bf, tag="a6")
            relu_evict(a6[...], h6[...])
            tph2 = sb(tpsum, [128, 128], bf, tag="tp")
            nc.tensor.transpose(tph2[0:128, 0:4], a6[...], id128h_sb[0:4, 0:4])
            a6T = sb(mpool, [128, 4], bf, tag="a6T")
            nc.vector.tensor_copy(a6T[...], tph2[0:128, 0:4])

            encp = sb(smallps, [nclg, lout], f32, tag="smallps")
            nc.tensor.matmul(encp[...], a6T[:, 0:nclg], how_sb[...], start=True, stop=True)
            return encp, s6

        def dec_ln_layer(zT_ap, w_sb, din_, dout_, nclg, s_prev=None):
            """One decoder LN layer: returns (act padded tile, s_l)."""
            dp = sb(smallps, [nclg, dout_], f32, tag="smallps")
            nc.tensor.matmul(dp[...], zT_ap, w_sb[...], start=True, stop=True)
            td = sb(stpool, [nclg, 1], f32, tag="hstat")
            sqs = sb(sqpool, [128, 512], bf, tag="sqs")
            nc.scalar.activation(sqs[0:nclg, 0:dout_], dp[...], AF.Square,
                                 accum_out=td[...])
            u = sb(stpool, [nclg, 1], f32, tag="hstat")
            if s_prev is None:
                nc.vector.tensor_scalar(u[...], td[...], 1.0 / float(dout_), EPS,
                                        OP.mult, OP.add)
            else:
                nc.vector.tensor_tensor(u[...], s_prev[...], s_prev[...], OP.mult)
                nc.vector.tensor_scalar_mul(u[...], u[...], 1.0 / float(dout_))
                nc.vector.tensor_tensor(u[...], u[...], td[...], OP.mult)
                nc.vector.tensor_scalar_add(u[...], u[...], EPS)
            w = sb(stpool, [nclg, 1], f32, tag="hstat")
            nc.scalar.activation(w[...], u[...], AF.Sqrt)
            r = sb(stpool, [nclg, 1], f32, tag="hstat")
            nc.vector.reciprocal(r[...], w[...])
            if s_prev is not None:
                nc.vector.tensor_tensor(r[...], s_prev[...], r[...], OP.mult)
            a = sb(mpool, [4, 128], bf, tag=f"da{dout_}")
            relu_evict(a[0:nclg, 0:dout_], dp[...])
            tpd = sb(tpsum, [128, 128], bf, tag="tp")
            nc.tensor.transpose(tpd[0:dout_, 0:4], a[0:nclg, 0:dout_],
                                id128h_sb[0:4, 0:4])
            aT = sb(mpool, [dout_, 4], bf, tag=f"daT{dout_}")
            nc.vector.tensor_copy(aT[...], tpd[0:dout_, 0:4])
            return aT, r

        # =================== t-branch (q) ===================
        tws = (lambda ci: tw0_sb[...], tw1_sb, tw2_sb, tw3_sb)
        # chunk ci = 4*pt + cl ; cloud cl uses pcT cols [cl*512 + pt*128]
        a4s_t, s4_t = conv_group(tws, lambda ci: (ci % 4) * N + (ci // 4) * 128)
        gT_t = [sb(mpool, [128, 4], bf, tag="gTt") for _ in range(4)]
        for cl in range(4):
            m = pool_cloud(a4s_t, s4_t, [4 * pt + cl for pt in range(4)])
            pool_reduce(m, gT_t, cl)
        encp_t, s6_t = head(gT_t, None, thw0_sb, thw1_sb, thow_sb, 4, 4)
        qraw = sb(mpool, [4, 4], f32, tag="qraw")
        nc.scalar.mul(qraw[...], encp_t[...], s6_t[...])
        # normalize q
        q2 = sb(mpool, [4, 4], f32, tag="q2")
        nc.vector.tensor_tensor(q2[...], qraw[...], qraw[...], OP.mult)
        nrm = sb(mpool, [4, 1], f32, tag="nrm")
        nc.vector.tensor_reduce(nrm[...], q2[...], mybir.AxisListType.X, OP.add)
        nq = sb(mpool, [4, 1], f32, tag="nq")
        nc.scalar.activation(nq[...], nrm[...], AF.Sqrt)
        inq = sb(mpool, [4, 1], f32, tag="inq")
        nc.vector.reciprocal(inq[...], nq[...])
        qn = sb(mpool, [4, 4], f32, tag="qn")
        nc.vector.tensor_scalar_mul(qn[...], qraw[...], inq[...])
        # products: cols [xy, xz, yz, wz, wy, wx, xx, yy, zz]
        prod = sb(mpool, [4, 9], f32, tag="prod")
        pairs = [(0, 1), (0, 2), (1, 2), (3, 2), (3, 1), (3, 0), (0, 0), (1, 1), (2, 2)]
        for i, (a_, b_) in enumerate(pairs):
            nc.vector.tensor_tensor(prod[:, i:i + 1], qn[:, a_:a_ + 1],
                                    qn[:, b_:b_ + 1], OP.mult)
        su = sb(mpool, [4, 3], f32, tag="su")
        nc.vector.tensor_tensor(su[:, 0:1], prod[:, 7:8], prod[:, 8:9], OP.add)
        nc.vector.tensor_tensor(su[:, 1:2], prod[:, 6:7], prod[:, 8:9], OP.add)
        nc.vector.tensor_tensor(su[:, 2:3], prod[:, 6:7], prod[:, 7:8], OP.add)
        Rq = sb(mpool, [4, 9], f32, tag="Rq")
        # diag entries e = 0,4,8 : 1 - 2*su
        nc.vector.tensor_scalar(Rq[:, 0:9:4], su[...], -2.0, 1.0, OP.mult, OP.add)
        # off-diag (e = 3j+d): R10->e1, R20->e2, R01->e3, R21->e5, R02->e6, R12->e7
        nc.vector.tensor_tensor(Rq[:, 1:2], prod[:, 0:1], prod[:, 3:4], OP.add)       # xy+wz
        nc.vector.tensor_tensor(Rq[:, 2:3], prod[:, 1:2], prod[:, 4:5], OP.subtract)  # xz-wy
        nc.vector.tensor_tensor(Rq[:, 3:4], prod[:, 0:1], prod[:, 3:4], OP.subtract)  # xy-wz
        nc.vector.tensor_tensor(Rq[:, 5:6], prod[:, 2:3], prod[:, 5:6], OP.add)       # yz+wx
        nc.vector.tensor_tensor(Rq[:, 6:7], prod[:, 1:2], prod[:, 4:5], OP.add)       # xz+wy
        nc.vector.tensor_tensor(Rq[:, 7:8], prod[:, 2:3], prod[:, 5:6], OP.subtract)  # yz-wx
        nc.vector.tensor_scalar_mul(Rq[:, 1:4], Rq[:, 1:4], 2.0)
        nc.vector.tensor_scalar_mul(Rq[:, 5:8], Rq[:, 5:8], 2.0)
        rqp = sb(smallps, [9, 4], f32, tag="smallps")
        nc.tensor.transpose(rqp[...], Rq[...], id4_sb[...])
        r9 = sb(mpool, [9, 4], f32, tag="r9")
        nc.scalar.copy(r9[...], rqp[...])
        for j in range(3):
            for d in range(3):
                nc.gpsimd.dma_start(
                    out=RT[j:j + 1, d:d + 10:3],
                    in_=r9[3 * j + d:3 * j + d + 1, 0:4])

        # =================== e-branch per local-b group (pipeline-2) ===========
        def e_conv(bl, mid_cb=None):
            ews = (lambda ci: ew0_sb[:, 64 * (ci % 4):64 * (ci % 4) + 64],
                   ew1_sb, ew2_sb, ew3_sb)
            return conv_group(ews, lambda ci: bl * N + (ci // 4) * 128, mid_cb=mid_cb)

        def e_tail(bl, a4s, s4):
            gTe = [sb(mpool, [128, 4], bf, tag="gTe") for _ in range(4)]
            for rr in range(4):
                m = pool_cloud(a4s, s4, [4 * pt + rr for pt in range(4)])
                pool_reduce(m, gTe, rr)
            encp, s6 = head(gTe, None, ehw0_sb, ehw1_sb, ehow_sb, 64, 4)
            encg = sb(mpool, [4, 64], f32, tag="encg")
            nc.scalar.mul(encg[...], encp[...], s6[...])
            nc.sync.dma_start(out=enc_all[4 * bl:4 * bl + 4, :], in_=encg[...])
            encbf = sb(mpool, [4, 64], bf, tag="encbf")
            nc.vector.tensor_copy(encbf[...], encg[...])
            tpz = sb(tpsum, [128, 128], bf, tag="tp")
            nc.tensor.transpose(tpz[0:64, 0:4], encbf[...], id128h_sb[0:4, 0:4])
            zT = sb(mpool, [64, 4], bf, tag="zT")
            nc.vector.tensor_copy(zT[...], tpz[0:64, 0:4])
            b1T, s7 = dec_ln_layer(zT[...], dw0_sb, 64, 16, 4)
            b2T, s8 = dec_ln_layer(b1T[...], dw1_sb, 16, 32, 4, s7)
            b3T, s9 = dec_ln_layer(b2T[...], dw2_sb, 32, 36, 4, s8)
            tmp3 = sb(mpool, [4, 3, 512], f32, tag="dtmp", bufs=2)
            tmp23 = sb(mpool, [4, 3, 512], f32, tag="dtmp2", bufs=2)
            for d in range(3):
                ddp = sb(smallps, [4, 512], f32, tag="smallps")
                nc.tensor.matmul(ddp[...], b3T[...], dw3_sb[:, 512 * d:512 * (d + 1)],
                                 start=True, stop=True)
                nc.scalar.mul(tmp3[:, d, :], ddp[...], s9[...])
                nc.vector.tensor_scalar_mul(tmp23[:, d, :], tmp3[:, d, :], -2.0)
            sqt = sb(mpool, [4, 512], f32, tag="sqt", bufs=2)
            scr = sb(mpool, [4, 512], f32, tag="scr", bufs=2)
            nc.vector.tensor_tensor(sqt[...], tmp3[:, 0, :], tmp3[:, 0, :], OP.mult)
            nc.vector.tensor_tensor(scr[...], tmp3[:, 1, :], tmp3[:, 1, :], OP.mult)
            nc.vector.tensor_tensor(sqt[...], sqt[...], scr[...], OP.add)
            nc.vector.tensor_tensor(scr[...], tmp3[:, 2, :], tmp3[:, 2, :], OP.mult)
            nc.vector.tensor_tensor(sqt[...], sqt[...], scr[...], OP.add)
            for cl in range(4):
                c = 4 * bl + cl
                PcL = sb(pcpool, [5, 512], f32, tag="PcL")
                PcR = sb(pcpool, [5, 512], f32, tag="PcR")
                nc.sync.dma_start(out=PcL[4:5, :], in_=ones512[...])
                nc.sync.dma_start(out=PcR[3:4, :], in_=ones512[...])
                nc.sync.dma_start(out=PcL[0:3, :], in_=tmp23[cl:cl + 1, :, :])
                nc.sync.dma_start(out=PcR[0:3, :], in_=tmp3[cl:cl + 1, :, :])
                nc.sync.dma_start(out=PcL[3:4, :], in_=sqt[cl:cl + 1, :])
                nc.sync.dma_start(out=PcR[4:5, :], in_=sqt[cl:cl + 1, :])
                for i in range(4):
                    Tp = sb(gramps, [128, 512], f32, tag="gramps")
                    nc.tensor.matmul(Tp[...], PcL[0:5, 128 * i:128 * (i + 1)],
                                     PcR[0:5, :], start=True, stop=True)
                    nc.vector.tensor_tensor(Tp[:, 128 * i:128 * (i + 1)],
                                            Tp[:, 128 * i:128 * (i + 1)],
                                            bigi_sb[...], OP.add)
                    nc.vector.tensor_reduce(minds[:, 4 * c + i:4 * c + i + 1], Tp[...],
                                            mybir.AxisListType.X, OP.min)
                rp = sb(smallps, [3, 512], f32, tag="smallps")
                nc.tensor.matmul(rp[...], RT[:, 3 * bl:3 * bl + 3], PcR[0:3, :],
                                 start=True, stop=True)
                rsb = sb(mpool, [3, 512], f32, tag="rsb", bufs=3)
                nc.scalar.copy(rsb[...], rp[...])
                nc.sync.dma_start(out=rot_out[c], in_=rsb[...])
            ovr = sb(mpool, [128, 16], f32, tag="ovr", bufs=2)
            nc.scalar.activation(ovr[...], minds[:, 16 * bl:16 * bl + 16], AF.Relu,
                                 bias=TWO_R, scale=-1.0)
            nc.vector.tensor_reduce(ovcols[:, bl:bl + 1], ovr[...],
                                    mybir.AxisListType.X, OP.add)

        prev = [None]

        def emit_prev_tail():
            if prev[0] is not None:
                e_tail(*prev[0])
                prev[0] = None

        for bl in range(BL):
            a4s, s4 = e_conv(bl, mid_cb=emit_prev_tail)
            prev[0] = (bl, a4s, s4)
        emit_prev_tail()

        # =================== losses ===================
        ovc = sb(mpool, [128, 1], f32, tag="ovc")
        nc.vector.tensor_reduce(ovc[...], ovcols[...], mybir.AxisListType.X, OP.add)
        ovp = sb(smallps, [1, 1], f32, tag="smallps")
        nc.tensor.matmul(ovp[...], ones_sb[...], ovc[...], start=True, stop=True)

        esp = sb(smallps, [4, 64], f32, tag="smallps")
        nc.tensor.matmul(esp[...], sel_sb[...], enc_all[...], start=True, stop=True)
        es = sb(mpool, [4, 64], f32, tag="es")
        nc.scalar.copy(es[...], esp[...])
        es2 = sb(mpool, [4, 64], f32, tag="es2")
        nc.vector.tensor_tensor(es2[...], es[...], es[...], OP.mult)
        b2c = sb(mpool, [4, 1], f32, tag="b2c")
        nc.vector.tensor_reduce(b2c[...], es2[...], mybir.AxisListType.X, OP.add)
        enc2 = sb(mpool, [16, 64], f32, tag="enc2")
        nc.vector.tensor_tensor(enc2[...], enc_all[...], enc_all[...], OP.mult)
        n2c = sb(mpool, [16, 1], f32, tag="n2c")
        nc.vector.tensor_reduce(n2c[...], enc2[...], mybir.AxisListType.X, OP.add)
        a2p = sb(smallps, [4, 1], f32, tag="smallps")
        nc.tensor.matmul(a2p[...], sel_sb[...], n2c[...], start=True, stop=True)
        crcc = sb(mpool, [4, 1], f32, tag="crcc")
        nc.scalar.mul(crcc[...], a2p[...], float(R))
        nc.vector.tensor_tensor(crcc[...], crcc[...], b2c[...], OP.subtract)
        crcp = sb(smallps, [1, 1], f32, tag="smallps")
        nc.tensor.matmul(crcp[...], ones_sb[0:4, 0:1], crcc[...], start=True, stop=True)

        loss_sb = sb(mpool, [1, 2], f32, tag="loss")
        nc.scalar.copy(loss_sb[0:1, 0:1], ovp[...])
        nc.scalar.copy(loss_sb[0:1, 1:2], crcp[...])
        nc.sync.dma_start(out=loss_out[...], in_=loss_sb[...])

    nc.compile()
    return nc


def _prep_weights(rotate_matrix, t_params, e_params, d_params):
    rm = _f32(rotate_matrix)

    def chk(p, keys0, keys1):
        for k in keys0:
            assert np.all(np.asarray(p[k]) == 0.0), f"{k} nonzero"
        for k in keys1:
            assert np.all(np.asarray(p[k]) == 1.0), f"{k} != 1"

    for p in (t_params, e_params):
        chk(p, [f"cb{i}" for i in range(4)] + [f"cbe{i}" for i in range(4)]
            + [f"db{i}" for i in range(2)] + [f"dbe{i}" for i in range(2)] + ["ob"],
            [f"cg{i}" for i in range(4)] + [f"dg{i}" for i in range(2)])
    chk(d_params, [f"b{i}" for i in range(3)] + [f"be{i}" for i in range(3)] + ["b3"],
        [f"g{i}" for i in range(3)])

    def conv_pack(p):
        w1c = _center(_f32(p["cw1"]))
        w1 = _bf(np.concatenate([w1c, w1c], axis=0))
        w2 = _bf(_center(_f32(p["cw2"])))
        w3 = _bf(_center(_f32(p["cw3"])).reshape(2, 128, 512).transpose(1, 0, 2))
        hw0 = _bf(_center(_f32(p["dw0"])).reshape(4, 128, 256).transpose(1, 0, 2))
        hw1 = _bf(_center(_f32(p["dw1"])).reshape(2, 128, 128).transpose(1, 0, 2))
        how = _bf(_f32(p["ow"]))
        return w1, w2, w3, hw0, hw1, how

    ew1, ew2, ew3, ehw0, ehw1, ehow = conv_pack(e_params)
    tw1, tw2, tw3, thw0, thw1, thow = conv_pack(t_params)
    ecw0 = _f32(e_params["cw0"])
    ew0 = _bf(np.concatenate([_center(rm[r].T @ ecw0) for r in range(R)], axis=1))
    tw0 = _bf(_center(_f32(t_params["cw0"])))
    dw0 = _bf(_center(_f32(d_params["w0"])))
    dw1 = _bf(_center(_f32(d_params["w1"])))
    dw2 = _bf(_center(_f32(d_params["w2"])))
    w3 = _f32(d_params["w3"])  # [36, 1536], cols j = n*3+d -> d-major
    dw3 = _bf(w3.reshape(36, 512, 3).transpose(0, 2, 1).reshape(36, 3 * 512))

    consts = {
        "bigi": np.eye(128, dtype=np.float32) * BIG,
        "sel": np.repeat(np.eye(4, dtype=np.float32), 4, axis=0),  # [16,4], c=4bl+r
        "id4": np.eye(4, dtype=np.float32),
        "id128h": np.eye(128, dtype=np.float16),
        "ide": np.array([[1.0 if q == p + 384 else 0.0 for q in range(896)]
                         for p in range(128)], np.float32),
        "onesh": np.ones((128, 1), np.float16),
        "sel2": np.concatenate([np.repeat([[1.0, 0.0]], 64, 0),
                                np.repeat([[0.0, 1.0]], 64, 0)]).astype(np.float16),
        "ones128": np.ones((128, 1), np.float32),
        "ones512": np.ones((1, 512), np.float32),
    }
    shared = dict(ew0=ew0, ew1=ew1, ew2=ew2, ew3=ew3, ehw0=ehw0, ehw1=ehw1,
                  ehow=ehow, tw0=tw0, tw1=tw1, tw2=tw2, tw3=tw3, thw0=thw0,
                  thw1=thw1, thow=thow, dw0=dw0, dw1=dw1, dw2=dw2, dw3=dw3,
                  **consts)
    return shared


def kernel(point_cloud, rotate_matrix, t_params, e_params, d_params):
    from concourse.bass_utils import run_bass_kernel_spmd

    if "nc" not in _CACHE:
        _CACHE["nc"] = _build_nc()
    nc = _CACHE["nc"]

    pc = _f32(point_cloud)
    shared = _prep_weights(rotate_matrix, t_params, e_params, d_params)

    in_maps = []
    for k in range(NCORES):
        pcl = pc[BL * k:BL * (k + 1)]                      # [4, 512, 3]
        pcT = _bf(pcl.transpose(2, 0, 1).reshape(3, BL * N))
        in_maps.append({"pcT": pcT, **shared})

    res = run_bass_kernel_spmd(nc, in_maps, core_ids=list(range(NCORES)))

    rotated = np.zeros((R, B, N, D), np.float32)
    ov_sum = 0.0
    crc_sum = 0.0
    for k in range(NCORES):
        rot = res.results[k]["rot_out"].reshape(BL, R, 3, N)     # [bl, r, d, n]
        rotated[:, BL * k:BL * (k + 1)] = rot.transpose(1, 0, 3, 2)
        loss = res.results[k]["loss_out"].reshape(2)
        ov_sum += float(loss[0])
        crc_sum += float(loss[1])

    npairs = R * (R - 1) // 2
    crc = np.float32(crc_sum / (npairs * B))
    overlap = np.float32(ov_sum / (R * B * N))
    return rotated, crc, overlap
